# revision 29
# baseline (speedup 1.0000x reference)
"""Trainium2 Bass kernel for masked attention with attention-probability output.

reference:
    scores = (Q @ K^T + dis) / 16, masked -> -inf, p_attn = softmax(scores),
    p_val = p_attn @ V;  returns (p_val, p_attn)

Shapes (hardcoded): B=16, N=1024, C=256, fp32. 8 NeuronCores, data-parallel
over batch (2 batches per core, no cross-device comm).

Host prep: qT/kT = q/k transposed to [C, N] (so the contraction dim C sits on
SBUF partitions); dism = dis/16 with masked entries -1e30, shipped as bf16
(mask folded in, halves the biggest input).

Per-core pipeline (per batch b, per 128-row q-chunk):
  - S = Q K^T via float32r matmuls (full PE rate; fp32 would be 4 cycles/row)
    accumulated fp32 in PSUM [128, 1024].
  - X = S*(1/16) + dism   (DVE scalar_tensor_tensor, in-place in PSUM).
  - expS = Exp(X) on ACT -> SBUF, DMA out fp32 (this is p_attn's numerator).
  - expS^T via PE transposes (fp32, exact) -> PSUM -> DVE rounding copy to
    f32r SBUF.
  - U = expS^T.T @ V via float32r matmuls accumulated over k in PSUM,
    copied to SBUF on ACT, DMA out fp32.
Softmax denominators are applied on the host (p_attn = expS/rowsum,
p_val = U/rowsum) - exact fp32 math with zero device cost and two fewer
dependency hops per chunk on the device.
"""

import numpy as np
import ml_dtypes

B = 16
N = 1024
C = 256
P = 128
N_CORES = 8
B_PER_CORE = B // N_CORES
SCALE = 1.0 / 16.0  # 1/sqrt(C)
MASK_NEG = -1e30

_cache = {}


def _build_program(repeat=1, **opt):
    import concourse.mybir as mybir
    import concourse.tile as tile
    from concourse import bacc
    from concourse.masks import make_identity

    o = {
        "dm_bufs": 6,
        "ex_bufs": 4,
        "xt_bufs": 4,
        "pv_bufs": 4,
        "ps_s_bufs": 2,
        "ps_t_bufs": 2,
        "ps_v_bufs": 2,
        "pval_engine": "scalar",  # scalar | vector
        "split_halves": False,
        "exps_f32r": False,  # ACT Exp writes f32r -> 1.5 c/r transposes
        "merge_copy": False,  # one [128,1024] PSUM->SBUF copy per chunk
        "dm_early": 3,  # how many dism chunk loads to hoist into batch loads
    }
    o.update(opt)

    f32 = mybir.dt.float32
    f32r = mybir.dt.float32r
    bf16 = mybir.dt.bfloat16

    nc = bacc.Bacc(None, target_bir_lowering=False, debug=False)

    if o.get("bench_io"):
        # Bench-only variant: inputs baked into the NEFF (loaded to HBM once
        # at model-load), outputs to internal DRAM + one tiny external output.
        # Per-call transfer through the axon tunnel becomes negligible, so
        # wall-clock repeat-difference can resolve the kernel's device time.
        rng = np.random.default_rng(0)
        qT = nc.inline_tensor(
            rng.standard_normal((B_PER_CORE, C, N)).astype(np.float32), name="qT"
        ).ap()
        kT = nc.inline_tensor(
            rng.standard_normal((B_PER_CORE, C, N)).astype(np.float32), name="kT"
        ).ap()
        v = nc.inline_tensor(
            rng.standard_normal((B_PER_CORE, N, C)).astype(np.float32), name="v"
        ).ap()
        dism = nc.inline_tensor(
            (rng.standard_normal((B_PER_CORE, N, N)) * SCALE).astype(
                ml_dtypes.bfloat16
            ),
            name="dism",
        ).ap()
        p_val = nc.dram_tensor("p_val", [B_PER_CORE, N, C], f32).ap()
        p_attn = nc.dram_tensor("p_attn", [B_PER_CORE, N, N], f32).ap()
        ok = nc.dram_tensor("ok", [P, 4], f32, kind="ExternalOutput").ap()
    else:
        qT = nc.dram_tensor("qT", [B_PER_CORE, C, N], f32, kind="ExternalInput").ap()
        kT = nc.dram_tensor("kT", [B_PER_CORE, C, N], f32, kind="ExternalInput").ap()
        v = nc.dram_tensor("v", [B_PER_CORE, N, C], f32, kind="ExternalInput").ap()
        dism = nc.dram_tensor(
            "dism", [B_PER_CORE, N, N], bf16, kind="ExternalInput"
        ).ap()
        p_val = nc.dram_tensor(
            "p_val", [B_PER_CORE, N, C], f32, kind="ExternalOutput"
        ).ap()
        p_attn = nc.dram_tensor(
            "p_attn", [B_PER_CORE, N, N], f32, kind="ExternalOutput"
        ).ap()

    NQ = N // P  # q-chunks per batch
    NK = N // P  # k-chunks per batch
    CC = C // P  # contraction chunks

    with tile.TileContext(nc) as tc:
        with (
            tc.tile_pool(name="consts", bufs=1) as consts,
            tc.tile_pool(name="qk", bufs=2) as qk_pool,
            tc.tile_pool(name="vp", bufs=2) as v_pool,
            tc.tile_pool(name="dm", bufs=o["dm_bufs"]) as dm_pool,
            tc.tile_pool(name="ex", bufs=o["ex_bufs"]) as ex_pool,
            tc.tile_pool(name="xt", bufs=o["xt_bufs"]) as xt_pool,
            tc.tile_pool(name="pv", bufs=o["pv_bufs"]) as pv_pool,
            tc.tile_pool(name="sm", bufs=8) as sm_pool,
            tc.tile_pool(name="ps_s", bufs=o["ps_s_bufs"], space="PSUM") as ps_s_pool,
            tc.tile_pool(name="ps_t", bufs=o["ps_t_bufs"], space="PSUM") as ps_t_pool,
            tc.tile_pool(name="ps_v", bufs=o["ps_v_bufs"], space="PSUM") as ps_v_pool,
        ):
            ident = consts.tile([P, P], f32)
            make_identity(nc, ident)

            for b in [b for _ in range(repeat) for b in range(B_PER_CORE)]:
                # chunked loads, ordered so chunk 0's operands land first
                qT_sb = qk_pool.tile([P, CC, N], f32r, tag="qT")
                kT_sb = qk_pool.tile([P, CC, N], f32r, tag="kT")
                v_sb = v_pool.tile([P, NK, C], f32r, tag="v")
                qT_src = qT[b].rearrange("(cc p) n -> p cc n", p=P).bitcast(f32r)
                kT_src = kT[b].rearrange("(cc p) n -> p cc n", p=P).bitcast(f32r)

                def load_kT(kh):
                    nc.sync.dma_start(
                        out=kT_sb[:, :, kh * 512 : (kh + 1) * 512],
                        in_=kT_src[:, :, kh * 512 : (kh + 1) * 512],
                    )

                def load_qT(qi):
                    nc.sync.dma_start(
                        out=qT_sb[:, :, qi * P : (qi + 1) * P],
                        in_=qT_src[:, :, qi * P : (qi + 1) * P],
                    )

                dm_tiles = {}

                def load_dm(qi):
                    dm_tiles[qi] = dm_pool.tile(
                        [P, N], mybir.dt.bfloat16, tag="dm", name="dm"
                    )
                    nc.sync.dma_start(
                        out=dm_tiles[qi], in_=dism[b, qi * P : (qi + 1) * P, :]
                    )

                # interleave so chunk 0/1's full operand sets land first
                n_early = o["dm_early"]
                load_kT(0)
                load_qT(0)
                if n_early > 0:
                    load_dm(0)
                load_kT(1)
                load_qT(1)
                for qi in range(1, n_early):
                    load_dm(qi)
                for qi in range(2, NQ):
                    load_qT(qi)
                for kh in range(2):
                    nc.sync.dma_start(
                        out=v_sb[:, kh * 4 : (kh + 1) * 4, :],
                        in_=v[b, kh * 512 : (kh + 1) * 512, :]
                        .rearrange("(kk p) c -> p kk c", p=P)
                        .bitcast(f32r),
                    )

                for qi in range(NQ):
                    if qi not in dm_tiles:
                        load_dm(qi)
                    dm = dm_tiles[qi]

                    # unnormalized expS; softmax denominators are applied on
                    # the host (p_attn = expS / rowsum, p_val = U / rowsum)
                    exps = ex_pool.tile(
                        [P, N], f32r if o["exps_f32r"] else mybir.dt.float32,
                        tag="exps",
                    )
                    exps_f = exps.bitcast(f32) if o["exps_f32r"] else exps
                    if o["split_halves"]:
                        for kh in range(2):
                            ps_s = ps_s_pool.tile(
                                [P, 512], mybir.dt.float32, tag="ps_s"
                            )
                            for cc in range(CC):
                                nc.tensor.matmul(
                                    ps_s,
                                    lhsT=qT_sb[:, cc, qi * P : (qi + 1) * P],
                                    rhs=kT_sb[:, cc, kh * 512 : (kh + 1) * 512],
                                    start=(cc == 0),
                                    stop=(cc == CC - 1),
                                )
                            # X = S/16 + dism (in place in PSUM)
                            nc.vector.scalar_tensor_tensor(
                                out=ps_s,
                                in0=ps_s,
                                scalar=SCALE,
                                in1=dm[:, kh * 512 : (kh + 1) * 512],
                                op0=mybir.AluOpType.mult,
                                op1=mybir.AluOpType.add,
                            )
                            nc.scalar.activation(
                                out=exps[:, kh * 512 : (kh + 1) * 512],
                                in_=ps_s,
                                func=mybir.ActivationFunctionType.Exp,
                                scale=1.0,
                            )
                    else:
                        ps_s = ps_s_pool.tile([P, N], mybir.dt.float32, tag="ps_s")
                        for kh in range(2):
                            for cc in range(CC):
                                nc.tensor.matmul(
                                    ps_s[:, kh * 512 : (kh + 1) * 512],
                                    lhsT=qT_sb[:, cc, qi * P : (qi + 1) * P],
                                    rhs=kT_sb[:, cc, kh * 512 : (kh + 1) * 512],
                                    start=(cc == 0),
                                    stop=(cc == CC - 1),
                                )
                        nc.vector.scalar_tensor_tensor(
                            out=ps_s,
                            in0=ps_s,
                            scalar=SCALE,
                            in1=dm,
                            op0=mybir.AluOpType.mult,
                            op1=mybir.AluOpType.add,
                        )
                        nc.scalar.activation(
                            out=exps,
                            in_=ps_s,
                            func=mybir.ActivationFunctionType.Exp,
                            scale=1.0,
                        )

                    nc.scalar.dma_start(
                        out=p_attn[b, qi * P : (qi + 1) * P, :], in_=exps_f
                    )

                    # ---- expS^T (PE transpose) -> f32r SBUF ----
                    tr_dt = f32r if o["exps_f32r"] else f32
                    tr_ident = ident.bitcast(f32r) if o["exps_f32r"] else ident
                    expT = xt_pool.tile([P, N], f32r, tag="expT")
                    if o["merge_copy"]:
                        ps_t = ps_t_pool.tile([P, N], tr_dt, tag="ps_t")
                        for kk in range(NK):
                            nc.tensor.transpose(
                                ps_t[:, kk * P : (kk + 1) * P],
                                exps[:, kk * P : (kk + 1) * P],
                                tr_ident,
                            )
                        nc.vector.tensor_copy(expT, ps_t.bitcast(f32))
                    else:
                        for kh in range(2):
                            ps_t = ps_t_pool.tile([P, 512], tr_dt, tag="ps_t")
                            for j in range(4):
                                nc.tensor.transpose(
                                    ps_t[:, j * P : (j + 1) * P],
                                    exps[:, (kh * 4 + j) * P : (kh * 4 + j + 1) * P],
                                    tr_ident,
                                )
                            nc.vector.tensor_copy(
                                expT[:, kh * 512 : (kh + 1) * 512], ps_t.bitcast(f32)
                            )

                    # ---- U = expS^T.T @ V (accumulate over k chunks) ----
                    ps_v = ps_v_pool.tile([P, C], mybir.dt.float32, tag="ps_v")
                    for kk in range(NK):
                        nc.tensor.matmul(
                            ps_v,
                            lhsT=expT[:, kk * P : (kk + 1) * P],
                            rhs=v_sb[:, kk, :],
                            start=(kk == 0),
                            stop=(kk == NK - 1),
                        )

                    # ---- U out (normalized on host) ----
                    pv_t = pv_pool.tile([P, C], mybir.dt.float32, tag="pv")
                    if o["pval_engine"] == "scalar":
                        nc.scalar.copy(pv_t, ps_v)
                    else:
                        nc.vector.tensor_copy(pv_t, ps_v)
                    nc.scalar.dma_start(
                        out=p_val[b, qi * P : (qi + 1) * P, :], in_=pv_t
                    )
                    last_pv = pv_t

            if o.get("bench_io"):
                nc.sync.dma_start(out=ok, in_=last_pv[:, 0:4])

    nc.compile()
    return nc


LAST_RESULTS = None


def kernel(query, key, value, mask, dis):
    import os

    # Under axon without the NTFF hook module, a stray BASS_TRACE=1 in the
    # environment would crash run_bass_kernel_spmd on an ImportError.
    from concourse._compat import axon_active

    if axon_active():
        try:
            from antenv.axon_hooks import get_axon_ntff_profile_hook  # noqa: F401
        except ImportError:
            os.environ["BASS_NEVER_TRACE"] = "1"

    from concourse.bass_utils import run_bass_kernel_spmd

    global LAST_RESULTS

    if "nc" not in _cache:
        _cache["nc"] = _build_program()
    nc = _cache["nc"]

    query = np.asarray(query, dtype=np.float32)
    key = np.asarray(key, dtype=np.float32)
    value = np.asarray(value, dtype=np.float32)
    mask = np.asarray(mask)
    dis = np.asarray(dis, dtype=np.float32)

    qT = np.ascontiguousarray(query.transpose(0, 2, 1))
    kT = np.ascontiguousarray(key.transpose(0, 2, 1))
    dism = np.where(mask, np.float32(MASK_NEG), dis * np.float32(SCALE)).astype(
        ml_dtypes.bfloat16
    )

    in_maps = []
    for c in range(N_CORES):
        s = slice(c * B_PER_CORE, (c + 1) * B_PER_CORE)
        in_maps.append(
            {
                "qT": np.ascontiguousarray(qT[s]),
                "kT": np.ascontiguousarray(kT[s]),
                "v": np.ascontiguousarray(value[s]),
                "dism": np.ascontiguousarray(dism[s]),
            }
        )

    res = run_bass_kernel_spmd(nc, in_maps, core_ids=list(range(N_CORES)))
    LAST_RESULTS = res

    u = np.concatenate([r["p_val"] for r in res.results], axis=0)
    es = np.concatenate([r["p_attn"] for r in res.results], axis=0)
    # softmax denominators applied host-side (exact fp32 math, no HW cost)
    denom = es.sum(axis=-1, keepdims=True, dtype=np.float32)
    p_attn = es / denom
    p_val = u / denom
    return p_val, p_attn
